# revision 1
# baseline (speedup 1.0000x reference)
"""HDC generic encoder kernel for 8 Trainium2 NeuronCores.

out[b,d] = sum_{w=0..56} K[w,d] * prod_{j=0..6} enc0[b, w+1+j, (d-(6-j)) mod D]

enc0[b,p,:] is a +/-1 table row selected by level-quantizing x[b,0,p].
Sharding: pure data parallel over batch, 8 batches per core.

Device pipeline per core:
1. 4 indirect row-gathers: R_i[128, 10007] <- extended table rows for
   (b = 2i + p//64, pos = p%64).  One offset per partition (the HW contract).
2. SBUF->SBUF redistribution DMAs into compute layout G[q = b_local*16+blk],
   where each of the 128 partitions holds 64 row-segments of its d-block
   (632 elems = 625 + 6 halo + 1, stride 640), so every rolled factor read
   G[(w+1+j)*SEG + d' + j] is an affine in-partition offset read.
3. Window products via a 5-op tensor_tensor tree (even offsets everywhere
   except the final even*odd combine which runs at 1x):
     E1[p,e]  = G[p*SEG+e]   * G[(p+2)*SEG+e+2]     (factor pairs j, j+2)
     E2[w,d'] = E1[w+1,d']   * E1[w+5,d'+4]         (factors 0,2,4,6)
     O3[w,e]  = E1[w+2,e]    * G[(w+6)*SEG+e+4]     (factors 1,3,5; e=d'+1)
     PROD     = E2[w,d']     * O3[w,d'+1]
     BD       = PROD * keys
4. log-tree adds over w (exact in bf16: +-1 sums <= 57), f32 accumulation
   across w-chunks, one DMA of the [128, 625] accumulator to out [8, 10000].
"""

import numpy as np

import concourse.bacc as bacc
import concourse.bass as bass
import concourse.mybir as mybir
from concourse.bass_utils import run_bass_kernel_spmd
from concourse.tile import TileContext

B, T, F, D = 64, 4, 64, 10000
NGRAMS = 7
W = F - NGRAMS  # 57 windows
NCORES = 8
BPC = B // NCORES  # 8 batches per core
MROWS, HROWS = 3000, 200
VROWS = MROWS + HROWS

NBLK = 16
BLKW = D // NBLK  # 625
SEG = 640  # per-position segment stride in G (even, room for 632)
SEGW = 632  # valid elems per segment (625 + 6 halo + 1)
ET = D + 7  # extended table row width (6 wrap + row + 1)
KW = 626  # keys/intermediate per-window stride (even)

W_E1 = 630
W_E2 = 626
W_O3 = 626
W_PR = 625

CHUNKS = [(0, 8), (8, 8), (16, 8), (24, 8), (32, 8), (40, 8), (48, 8), (56, 1)]

_CACHE = {}


def _build_nc():
    nc = bacc.Bacc(None)
    tbl = nc.dram_tensor("tbl", [VROWS, ET], mybir.dt.bfloat16, kind="ExternalInput")
    keys2 = nc.dram_tensor(
        "keys2", [128, W * KW], mybir.dt.bfloat16, kind="ExternalInput"
    )
    goff = nc.dram_tensor("goff", [128, 4], mybir.dt.int32, kind="ExternalInput")
    out = nc.dram_tensor("out", [BPC, D], mybir.dt.float32, kind="ExternalOutput")
    out_r = out.rearrange("b (q d) -> (b q) d", d=BLKW)  # [128, 625]

    with TileContext(nc) as tc:
        with (
            tc.tile_pool(name="big", bufs=1) as bpool,
            tc.tile_pool(name="rp", bufs=1) as rpool,
            tc.tile_pool(name="work", bufs=1) as wpool,
            tc.tile_pool(name="keysp", bufs=2) as kpool,
        ):
            goff_t = bpool.tile([128, 4], mybir.dt.int32, tag="goff")
            nc.sync.dma_start(out=goff_t[:, :], in_=goff[:, :])

            g = bpool.tile([128, F * SEG], mybir.dt.bfloat16, tag="G")
            g3 = g[:, :].rearrange("p (s k) -> p s k", k=SEG)
            for i in range(4):
                r = rpool.tile([128, ET], mybir.dt.bfloat16, tag="R", name=f"R{i}")
                nc.gpsimd.indirect_dma_start(
                    out=r[:, :],
                    out_offset=None,
                    in_=tbl[:, :],
                    in_offset=bass.IndirectOffsetOnAxis(ap=goff_t[:, i : i + 1], axis=0),
                )
                for h in range(2):
                    for blk in range(NBLK):
                        p0 = (2 * i + h) * NBLK + blk
                        nc.sync.dma_start(
                            out=g3[p0 : p0 + 1, :, 0:SEGW],
                            in_=r[h * 64 : (h + 1) * 64, blk * BLKW : blk * BLKW + SEGW],
                        )

            acc = bpool.tile([128, KW], mybir.dt.float32, tag="acc")
            nc.vector.memset(acc[:, :], 0.0)

            def rv(tile, stride, cnt, off, width):
                """Strided-row view: rows of `width` elems at `off + i*stride`."""
                base = (off // stride) * stride
                o2 = off - base
                v = tile[:, base : base + cnt * stride].rearrange(
                    "p (s k) -> p s k", k=stride
                )
                return v[:, :, o2 : o2 + width]

            for w0, wc in CHUNKS:
                kc = kpool.tile([128, wc * KW], mybir.dt.bfloat16, tag="kc")
                nc.sync.dma_start(out=kc[:, :], in_=keys2[:, w0 * KW : (w0 + wc) * KW])

                ne1 = wc + 4
                e1 = wpool.tile([128, ne1 * W_E1], mybir.dt.bfloat16, tag="e1")
                nc.vector.tensor_mul(
                    rv(e1, W_E1, ne1, 0, W_E1),
                    rv(g, SEG, ne1, (w0 + 1) * SEG, W_E1),
                    rv(g, SEG, ne1, (w0 + 3) * SEG + 2, W_E1),
                )
                e2 = wpool.tile([128, wc * KW], mybir.dt.bfloat16, tag="e2")
                nc.vector.tensor_mul(
                    rv(e2, KW, wc, 0, W_E2),
                    rv(e1, W_E1, wc, 0, W_E2),
                    rv(e1, W_E1, wc, 4 * W_E1 + 4, W_E2),
                )
                o3 = wpool.tile([128, wc * KW], mybir.dt.bfloat16, tag="o3")
                nc.vector.tensor_mul(
                    rv(o3, KW, wc, 0, W_O3),
                    rv(e1, W_E1, wc, W_E1, W_O3),
                    rv(g, SEG, wc, (w0 + 6) * SEG + 4, W_O3),
                )
                pr = wpool.tile([128, wc * KW], mybir.dt.bfloat16, tag="pr")
                nc.vector.tensor_mul(
                    rv(pr, KW, wc, 0, W_PR),
                    rv(e2, KW, wc, 0, W_PR),
                    rv(o3, KW, wc, 1, W_PR),
                )
                bd = wpool.tile([128, wc * KW], mybir.dt.bfloat16, tag="bd")
                nc.vector.tensor_mul(
                    rv(bd, KW, wc, 0, W_PR),
                    rv(pr, KW, wc, 0, W_PR),
                    rv(kc, KW, wc, 0, W_PR),
                )
                n = wc
                while n > 1:
                    m = n // 2
                    nc.vector.tensor_add(
                        rv(bd, KW, m, 0, W_PR),
                        rv(bd, KW, m, 0, W_PR),
                        rv(bd, KW, m, (n - m) * KW, W_PR),
                    )
                    n -= m
                nc.vector.tensor_add(acc[:, 0:W_PR], acc[:, 0:W_PR], bd[:, 0:W_PR])

            nc.sync.dma_start(out=out_r[:, :], in_=acc[:, 0:BLKW])
    nc.compile()
    return nc


def _host_prep(x, keys_weight, motion_table, hr_table):
    import ml_dtypes

    bf16 = ml_dtypes.bfloat16

    import jax.numpy as jnp

    x0 = jnp.asarray(x[:, 0, :])  # [B, F]
    mi = jnp.round((x0[:, : F - 1] - (-3.0)) / (3.0 - (-3.0)) * (MROWS - 1)).astype(
        jnp.int32
    )
    mi = jnp.clip(mi, 0, MROWS - 1)
    hi = jnp.round((x0[:, F - 1] - 50.0) / (200.0 - 50.0) * (HROWS - 1)).astype(
        jnp.int32
    )
    hi = jnp.clip(hi, 0, HROWS - 1) + MROWS
    rows = np.concatenate([np.asarray(mi), np.asarray(hi)[:, None]], axis=1).astype(
        np.int32
    )  # [B, F]

    tb = np.concatenate(
        [np.asarray(motion_table), np.asarray(hr_table)], axis=0
    ).astype(bf16)  # [VROWS, D]
    tbl = np.zeros((VROWS, ET), dtype=bf16)
    tbl[:, 0:6] = tb[:, D - 6 :]
    tbl[:, 6 : 6 + D] = tb
    tbl[:, 6 + D] = tb[:, 0]

    kb = np.asarray(keys_weight)[:W].astype(bf16)  # [57, D]
    karr = np.zeros((NBLK, W, KW), dtype=bf16)
    karr[:, :, :BLKW] = kb.reshape(W, NBLK, BLKW).transpose(1, 0, 2)
    keys2 = np.tile(karr.reshape(NBLK, W * KW), (BPC, 1))  # [128, W*KW]

    in_maps = []
    for c in range(NCORES):
        r8 = rows[BPC * c : BPC * (c + 1)]  # [8, F]
        # goff[p, i] = row index for batch 2i + p//64, pos p%64
        gof = r8.reshape(4, 2 * F).T.copy().astype(np.int32)  # [128, 4]
        in_maps.append({"tbl": tbl, "keys2": keys2, "goff": gof})
    return in_maps


def run(inputs, trace=False):
    if "nc" not in _CACHE:
        _CACHE["nc"] = _build_nc()
    nc = _CACHE["nc"]
    in_maps = _host_prep(**inputs)
    res = run_bass_kernel_spmd(nc, in_maps, core_ids=list(range(NCORES)), trace=trace)
    outs = [res.results[c]["out"] for c in range(NCORES)]
    full = np.concatenate(outs, axis=0).astype(np.float32)
    return full, res


def kernel(**inputs) -> np.ndarray:
    full, _ = run(inputs, trace=False)
    return full



# revision 5
# speedup vs baseline: 1.9077x; 1.9077x over previous
"""HDC generic encoder kernel for 8 Trainium2 NeuronCores.

out[b,d] = sum_{w=0..56} K[w,d] * prod_{j=0..6} enc0[b, w+1+j, (d-(6-j)) mod D]

enc0[b,p,:] is a +/-1 table row selected by level-quantizing x[b,0,p].
Sharding: pure data parallel over batch, 8 batches per core.

Device pipeline per core:
1. 4 indirect row-gathers: R_i[128, 10007] <- extended table rows for
   (b = 2i + p//64, pos = p%64).  One offset per partition (the HW contract).
2. SBUF->SBUF redistribution DMAs into compute layout G[q = b_local*16+blk],
   where each of the 128 partitions holds 64 row-segments of its d-block
   (632 elems = 625 + 6 halo + 1, stride 640), so every rolled factor read
   G[(w+1+j)*SEG + d' + j] is an affine in-partition offset read.
3. Window products via a 5-op tensor_tensor tree (even offsets everywhere
   except the final even*odd combine which runs at 1x):
     E1[p,e]  = G[p*SEG+e]   * G[(p+2)*SEG+e+2]     (factor pairs j, j+2)
     E2[w,d'] = E1[w+1,d']   * E1[w+5,d'+4]         (factors 0,2,4,6)
     O3[w,e]  = E1[w+2,e]    * G[(w+6)*SEG+e+4]     (factors 1,3,5; e=d'+1)
     PROD     = E2[w,d']     * O3[w,d'+1]
     BD       = PROD * keys
4. log-tree adds over w (exact in bf16: +-1 sums <= 57), f32 accumulation
   across w-chunks, one DMA of the [128, 625] accumulator to out [8, 10000].
"""

import numpy as np

import concourse.bacc as bacc
import concourse.bass as bass
import concourse.mybir as mybir
from concourse.bass_utils import run_bass_kernel_spmd
from concourse.tile import TileContext

B, T, F, D = 64, 4, 64, 10000
NGRAMS = 7
W = F - NGRAMS  # 57 windows
NCORES = 8
BPC = B // NCORES  # 8 batches per core
MROWS, HROWS = 3000, 200
VROWS = MROWS + HROWS

NBLK = 16
BLKW = D // NBLK  # 625
SEG = 640  # per-position segment stride in G (even, room for 632)
SEGW = 632  # valid elems per segment (625 + 6 halo + 1)
ET = D + 7  # extended table row width (6 wrap + row + 1)
KW = 626  # keys/intermediate per-window stride (even)

W_E1 = 630
W_E2 = 626
W_O3 = 626
W_PR = 625

CHUNKS = [(0, 8), (8, 8), (16, 8), (24, 8), (32, 8), (40, 8), (48, 8), (56, 1)]

_CACHE = {}


def _build_nc():
    nc = bacc.Bacc(None)
    tbl = nc.dram_tensor(
        "tbl", [VROWS, NBLK * SEG], mybir.dt.bfloat16, kind="ExternalInput"
    )
    keys2 = nc.dram_tensor(
        "keys2", [128, W * KW], mybir.dt.bfloat16, kind="ExternalInput"
    )
    goff = nc.dram_tensor("goff", [128, 4], mybir.dt.int32, kind="ExternalInput")
    out = nc.dram_tensor("out", [BPC, D], mybir.dt.float32, kind="ExternalOutput")
    out_r = out.rearrange("b (q d) -> (b q) d", d=BLKW)  # [128, 625]

    with TileContext(nc) as tc:
        with (
            tc.tile_pool(name="big", bufs=1) as bpool,
            tc.tile_pool(name="rp", bufs=1) as rpool,
            tc.tile_pool(name="work", bufs=1) as wpool,
            tc.tile_pool(name="keysp", bufs=2) as kpool,
        ):
            goff_t = bpool.tile([128, 4], mybir.dt.int32, tag="goff")
            nc.sync.dma_start(out=goff_t[:, :], in_=goff[:, :])

            g = bpool.tile([128, F * SEG], mybir.dt.bfloat16, tag="G")
            g3 = g[:, :].rearrange("p (s k) -> p s k", k=SEG)
            with tc.tile_pool(name="scr", bufs=2, space="DRAM") as dpool:
                for i in range(4):
                    # rows pre-blocked in HBM: r[p, blk*SEG+e] = ext_row_p[blk*BLKW+e]
                    r = rpool.tile(
                        [128, NBLK * SEG], mybir.dt.bfloat16, tag="R", name=f"R{i}"
                    )
                    nc.gpsimd.indirect_dma_start(
                        out=r[:, :],
                        out_offset=None,
                        in_=tbl[:, :],
                        in_offset=bass.IndirectOffsetOnAxis(
                            ap=goff_t[:, i : i + 1], axis=0
                        ),
                    )
                    # bounce via DRAM: rows out fat+contiguous, then one strided
                    # read back lands [32 partitions, 64 pos, 632] in G layout
                    scr = dpool.tile(
                        [128, NBLK * SEG], mybir.dt.bfloat16, tag="scr", name=f"S{i}"
                    )
                    nc.sync.dma_start(out=scr[:, :], in_=r[:, :])
                    for h in range(2):
                        p0 = (2 * i + h) * NBLK
                        sv = scr[h * F : (h + 1) * F, :].rearrange(
                            "s (blk e) -> blk s e", e=SEG
                        )
                        nc.sync.dma_start(
                            out=g3[p0 : p0 + NBLK, :, 0:SEGW].opt(),
                            in_=sv[:, :, 0:SEGW].opt(),
                        )

            acc = bpool.tile([128, KW], mybir.dt.float32, tag="acc")
            nc.vector.memset(acc[:, :], 0.0)

            def rv(tile, stride, cnt, off, width):
                """Strided-row view: rows of `width` elems at `off + i*stride`."""
                base = (off // stride) * stride
                o2 = off - base
                v = tile[:, base : base + cnt * stride].rearrange(
                    "p (s k) -> p s k", k=stride
                )
                return v[:, :, o2 : o2 + width]

            for w0, wc in CHUNKS:
                kc = kpool.tile([128, wc * KW], mybir.dt.bfloat16, tag="kc")
                nc.sync.dma_start(out=kc[:, :], in_=keys2[:, w0 * KW : (w0 + wc) * KW])

                ne1 = wc + 4
                e1 = wpool.tile([128, ne1 * W_E1], mybir.dt.bfloat16, tag="e1")
                nc.vector.tensor_mul(
                    rv(e1, W_E1, ne1, 0, W_E1),
                    rv(g, SEG, ne1, (w0 + 1) * SEG, W_E1),
                    rv(g, SEG, ne1, (w0 + 3) * SEG + 2, W_E1),
                )
                e2 = wpool.tile([128, wc * KW], mybir.dt.bfloat16, tag="e2")
                nc.vector.tensor_mul(
                    rv(e2, KW, wc, 0, W_E2),
                    rv(e1, W_E1, wc, 0, W_E2),
                    rv(e1, W_E1, wc, 4 * W_E1 + 4, W_E2),
                )
                o3 = wpool.tile([128, wc * KW], mybir.dt.bfloat16, tag="o3")
                nc.vector.tensor_mul(
                    rv(o3, KW, wc, 0, W_O3),
                    rv(e1, W_E1, wc, W_E1, W_O3),
                    rv(g, SEG, wc, (w0 + 6) * SEG + 4, W_O3),
                )
                pr = wpool.tile([128, wc * KW], mybir.dt.bfloat16, tag="pr")
                nc.vector.tensor_mul(
                    rv(pr, KW, wc, 0, W_PR),
                    rv(e2, KW, wc, 0, W_PR),
                    rv(o3, KW, wc, 1, W_PR),
                )
                bd = wpool.tile([128, wc * KW], mybir.dt.bfloat16, tag="bd")
                nc.vector.tensor_mul(
                    rv(bd, KW, wc, 0, W_PR),
                    rv(pr, KW, wc, 0, W_PR),
                    rv(kc, KW, wc, 0, W_PR),
                )
                n = wc
                while n > 1:
                    m = n // 2
                    nc.vector.tensor_add(
                        rv(bd, KW, m, 0, W_PR),
                        rv(bd, KW, m, 0, W_PR),
                        rv(bd, KW, m, (n - m) * KW, W_PR),
                    )
                    n -= m
                nc.vector.tensor_add(acc[:, 0:W_PR], acc[:, 0:W_PR], bd[:, 0:W_PR])

            nc.sync.dma_start(out=out_r[:, :], in_=acc[:, 0:BLKW])
    nc.compile()
    return nc


def _host_prep(x, keys_weight, motion_table, hr_table):
    import ml_dtypes

    bf16 = ml_dtypes.bfloat16

    import jax.numpy as jnp

    x0 = jnp.asarray(x[:, 0, :])  # [B, F]
    mi = jnp.round((x0[:, : F - 1] - (-3.0)) / (3.0 - (-3.0)) * (MROWS - 1)).astype(
        jnp.int32
    )
    mi = jnp.clip(mi, 0, MROWS - 1)
    hi = jnp.round((x0[:, F - 1] - 50.0) / (200.0 - 50.0) * (HROWS - 1)).astype(
        jnp.int32
    )
    hi = jnp.clip(hi, 0, HROWS - 1) + MROWS
    rows = np.concatenate([np.asarray(mi), np.asarray(hi)[:, None]], axis=1).astype(
        np.int32
    )  # [B, F]

    tb = np.concatenate(
        [np.asarray(motion_table), np.asarray(hr_table)], axis=0
    ).astype(bf16)  # [VROWS, D]
    ext = np.zeros((VROWS, ET), dtype=bf16)
    ext[:, 0:6] = tb[:, D - 6 :]
    ext[:, 6 : 6 + D] = tb
    ext[:, 6 + D] = tb[:, 0]
    # pre-blocked rows with halo baked in: tbl[r, blk*SEG + e] = ext[r, blk*BLKW + e]
    win = np.lib.stride_tricks.sliding_window_view(ext, SEGW, axis=1)  # [V, ET-631, 632]
    tbl = np.zeros((VROWS, NBLK, SEG), dtype=bf16)
    tbl[:, :, 0:SEGW] = win[:, 0 : NBLK * BLKW : BLKW, :]
    tbl = tbl.reshape(VROWS, NBLK * SEG)

    kb = np.asarray(keys_weight)[:W].astype(bf16)  # [57, D]
    karr = np.zeros((NBLK, W, KW), dtype=bf16)
    karr[:, :, :BLKW] = kb.reshape(W, NBLK, BLKW).transpose(1, 0, 2)
    keys2 = np.tile(karr.reshape(NBLK, W * KW), (BPC, 1))  # [128, W*KW]

    in_maps = []
    for c in range(NCORES):
        r8 = rows[BPC * c : BPC * (c + 1)]  # [8, F]
        # goff[p, i] = row index for batch 2i + p//64, pos p%64
        gof = r8.reshape(4, 2 * F).T.copy().astype(np.int32)  # [128, 4]
        in_maps.append({"tbl": tbl, "keys2": keys2, "goff": gof})
    return in_maps


def run(inputs, trace=False):
    if "nc" not in _CACHE:
        _CACHE["nc"] = _build_nc()
    nc = _CACHE["nc"]
    in_maps = _host_prep(**inputs)
    res = run_bass_kernel_spmd(nc, in_maps, core_ids=list(range(NCORES)), trace=trace)
    outs = [res.results[c]["out"] for c in range(NCORES)]
    full = np.concatenate(outs, axis=0).astype(np.float32)
    return full, res


def kernel(**inputs) -> np.ndarray:
    full, _ = run(inputs, trace=False)
    return full



# revision 9
# speedup vs baseline: 1.9705x; 1.0329x over previous
"""HDC generic encoder kernel for 8 Trainium2 NeuronCores.

out[b,d] = sum_{w=0..56} K[w,d] * prod_{j=0..6} enc0[b, w+1+j, (d-(6-j)) mod D]

enc0[b,p,:] is a +/-1 table row selected by level-quantizing x[b,0,p].
Sharding: pure data parallel over batch, 8 batches per core.

Device pipeline per core:
1. 4 indirect row-gathers: R_i[128, 10007] <- extended table rows for
   (b = 2i + p//64, pos = p%64).  One offset per partition (the HW contract).
2. SBUF->SBUF redistribution DMAs into compute layout G[q = b_local*16+blk],
   where each of the 128 partitions holds 64 row-segments of its d-block
   (632 elems = 625 + 6 halo + 1, stride 640), so every rolled factor read
   G[(w+1+j)*SEG + d' + j] is an affine in-partition offset read.
3. Window products via a 5-op tensor_tensor tree (even offsets everywhere
   except the final even*odd combine which runs at 1x):
     E1[p,e]  = G[p*SEG+e]   * G[(p+2)*SEG+e+2]     (factor pairs j, j+2)
     E2[w,d'] = E1[w+1,d']   * E1[w+5,d'+4]         (factors 0,2,4,6)
     O3[w,e]  = E1[w+2,e]    * G[(w+6)*SEG+e+4]     (factors 1,3,5; e=d'+1)
     PROD     = E2[w,d']     * O3[w,d'+1]
     BD       = PROD * keys
4. log-tree adds over w (exact in bf16: +-1 sums <= 57), f32 accumulation
   across w-chunks, one DMA of the [128, 625] accumulator to out [8, 10000].
"""

import numpy as np

import concourse.bacc as bacc
import concourse.bass as bass
import concourse.mybir as mybir
from concourse.bass_utils import run_bass_kernel_spmd
from concourse.tile import TileContext

B, T, F, D = 64, 4, 64, 10000
NGRAMS = 7
W = F - NGRAMS  # 57 windows
NCORES = 8
BPC = B // NCORES  # 8 batches per core
MROWS, HROWS = 3000, 200
VROWS = MROWS + HROWS

NBLK = 16
BLKW = D // NBLK  # 625
SEG = 640  # per-position segment stride in G (even, room for 632)
SEGW = 632  # valid elems per segment (625 + 6 halo + 1)
ET = D + 7  # extended table row width (6 wrap + row + 1)
KW = 626  # keys/intermediate per-window stride (even)

W_E1 = 630
W_E2 = 626
W_O3 = 626
W_PR = 625

CHUNKS = [(0, 8), (8, 8), (16, 8), (24, 8), (32, 8), (40, 8), (48, 8), (56, 1)]

_CACHE = {}


def _build_nc():
    nc = bacc.Bacc(None)
    tbl = nc.dram_tensor(
        "tbl", [VROWS, NBLK * SEG], mybir.dt.bfloat16, kind="ExternalInput"
    )
    keys2 = nc.dram_tensor(
        "keys2", [128, W * KW], mybir.dt.bfloat16, kind="ExternalInput"
    )
    goff = nc.dram_tensor("goff", [128, 4], mybir.dt.int32, kind="ExternalInput")
    out = nc.dram_tensor("out", [BPC, D], mybir.dt.float32, kind="ExternalOutput")
    out_r = out.rearrange("b (q d) -> (b q) d", d=BLKW)  # [128, 625]

    with TileContext(nc) as tc:
        with (
            tc.tile_pool(name="big", bufs=1) as bpool,
            tc.tile_pool(name="rp", bufs=2) as rpool,
            tc.tile_pool(name="work", bufs=1) as wpool,
            tc.tile_pool(name="keysp", bufs=2) as kpool,
        ):
            goff_t = bpool.tile([128, 4], mybir.dt.int32, tag="goff")
            nc.sync.dma_start(out=goff_t[:, :], in_=goff[:, :])

            g = bpool.tile([128, F * SEG], mybir.dt.bfloat16, tag="G")
            g3 = g[:, :].rearrange("p (s k) -> p s k", k=SEG)
            with tc.tile_pool(name="scr", bufs=2, space="DRAM") as dpool:
                # order alternates even/odd SDMA engine groups so hop2s overlap
                for i in (0, 2, 1, 3):
                    # rows pre-blocked in HBM: r[p, blk*SEG+e] = ext_row_p[blk*BLKW+e]
                    r = rpool.tile(
                        [128, NBLK * SEG], mybir.dt.bfloat16, tag="R", name=f"R{i}"
                    )
                    nc.gpsimd.indirect_dma_start(
                        out=r[:, :],
                        out_offset=None,
                        in_=tbl[:, :],
                        in_offset=bass.IndirectOffsetOnAxis(
                            ap=goff_t[:, i : i + 1], axis=0
                        ),
                    )
                    # bounce via DRAM: rows out fat+contiguous, then one strided
                    # read back lands [32 partitions, 64 pos, 632] in G layout
                    scr = dpool.tile(
                        [128, NBLK * SEG], mybir.dt.bfloat16, tag="scr", name=f"S{i}"
                    )
                    nc.sync.dma_start(out=scr[:, :], in_=r[:, :])
                    for h in range(2):
                        p0 = (2 * i + h) * NBLK
                        sv = scr[h * F : (h + 1) * F, :].rearrange(
                            "s (blk e) -> blk s e", e=SEG
                        )
                        nc.sync.dma_start(
                            out=g3[p0 : p0 + NBLK, :, 0:SEGW].opt(),
                            in_=sv[:, :, 0:SEGW].opt(),
                        )

            acc = bpool.tile([128, KW], mybir.dt.float32, tag="acc")
            nc.vector.memset(acc[:, :], 0.0)

            def rv(tile, stride, cnt, off, width):
                """Strided-row view: rows of `width` elems at `off + i*stride`."""
                base = (off // stride) * stride
                o2 = off - base
                v = tile[:, base : base + cnt * stride].rearrange(
                    "p (s k) -> p s k", k=stride
                )
                return v[:, :, o2 : o2 + width]

            for w0, wc in CHUNKS:
                kc = kpool.tile([128, wc * KW], mybir.dt.bfloat16, tag="kc")
                nc.sync.dma_start(out=kc[:, :], in_=keys2[:, w0 * KW : (w0 + wc) * KW])

                ne1 = wc + 4
                e1 = wpool.tile([128, ne1 * W_E1], mybir.dt.bfloat16, tag="e1")
                nc.vector.tensor_mul(
                    rv(e1, W_E1, ne1, 0, W_E1),
                    rv(g, SEG, ne1, (w0 + 1) * SEG, W_E1),
                    rv(g, SEG, ne1, (w0 + 3) * SEG + 2, W_E1),
                )
                e2 = wpool.tile([128, wc * KW], mybir.dt.bfloat16, tag="e2")
                nc.vector.tensor_mul(
                    rv(e2, KW, wc, 0, W_E2),
                    rv(e1, W_E1, wc, 0, W_E2),
                    rv(e1, W_E1, wc, 4 * W_E1 + 4, W_E2),
                )
                o3 = wpool.tile([128, wc * KW], mybir.dt.bfloat16, tag="o3")
                nc.vector.tensor_mul(
                    rv(o3, KW, wc, 0, W_O3),
                    rv(e1, W_E1, wc, W_E1, W_O3),
                    rv(g, SEG, wc, (w0 + 6) * SEG + 4, W_O3),
                )
                pr = wpool.tile([128, wc * KW], mybir.dt.bfloat16, tag="e1", name="pr")
                nc.vector.tensor_mul(
                    rv(pr, KW, wc, 0, W_PR),
                    rv(e2, KW, wc, 0, W_PR),
                    rv(o3, KW, wc, 1, W_PR),
                )
                bd = wpool.tile([128, wc * KW], mybir.dt.bfloat16, tag="e2", name="bd")
                nc.vector.tensor_mul(
                    rv(bd, KW, wc, 0, W_PR),
                    rv(pr, KW, wc, 0, W_PR),
                    rv(kc, KW, wc, 0, W_PR),
                )
                n = wc
                while n > 1:
                    m = n // 2
                    nc.vector.tensor_add(
                        rv(bd, KW, m, 0, W_PR),
                        rv(bd, KW, m, 0, W_PR),
                        rv(bd, KW, m, (n - m) * KW, W_PR),
                    )
                    n -= m
                nc.vector.tensor_add(acc[:, 0:W_PR], acc[:, 0:W_PR], bd[:, 0:W_PR])

            nc.sync.dma_start(out=out_r[:, :], in_=acc[:, 0:BLKW])
    nc.compile()
    return nc


def _host_prep(x, keys_weight, motion_table, hr_table):
    import ml_dtypes

    bf16 = ml_dtypes.bfloat16

    import jax.numpy as jnp

    x0 = jnp.asarray(x[:, 0, :])  # [B, F]
    mi = jnp.round((x0[:, : F - 1] - (-3.0)) / (3.0 - (-3.0)) * (MROWS - 1)).astype(
        jnp.int32
    )
    mi = jnp.clip(mi, 0, MROWS - 1)
    hi = jnp.round((x0[:, F - 1] - 50.0) / (200.0 - 50.0) * (HROWS - 1)).astype(
        jnp.int32
    )
    hi = jnp.clip(hi, 0, HROWS - 1) + MROWS
    rows = np.concatenate([np.asarray(mi), np.asarray(hi)[:, None]], axis=1).astype(
        np.int32
    )  # [B, F]

    tb = np.concatenate(
        [np.asarray(motion_table), np.asarray(hr_table)], axis=0
    ).astype(bf16)  # [VROWS, D]
    ext = np.zeros((VROWS, ET), dtype=bf16)
    ext[:, 0:6] = tb[:, D - 6 :]
    ext[:, 6 : 6 + D] = tb
    ext[:, 6 + D] = tb[:, 0]
    # pre-blocked rows with halo baked in: tbl[r, blk*SEG + e] = ext[r, blk*BLKW + e]
    win = np.lib.stride_tricks.sliding_window_view(ext, SEGW, axis=1)  # [V, ET-631, 632]
    tbl = np.zeros((VROWS, NBLK, SEG), dtype=bf16)
    tbl[:, :, 0:SEGW] = win[:, 0 : NBLK * BLKW : BLKW, :]
    tbl = tbl.reshape(VROWS, NBLK * SEG)

    kb = np.asarray(keys_weight)[:W].astype(bf16)  # [57, D]
    karr = np.zeros((NBLK, W, KW), dtype=bf16)
    karr[:, :, :BLKW] = kb.reshape(W, NBLK, BLKW).transpose(1, 0, 2)
    keys2 = np.tile(karr.reshape(NBLK, W * KW), (BPC, 1))  # [128, W*KW]

    in_maps = []
    for c in range(NCORES):
        r8 = rows[BPC * c : BPC * (c + 1)]  # [8, F]
        # goff[p, i] = row index for batch 2i + p//64, pos p%64
        gof = r8.reshape(4, 2 * F).T.copy().astype(np.int32)  # [128, 4]
        in_maps.append({"tbl": tbl, "keys2": keys2, "goff": gof})
    return in_maps


def run(inputs, trace=False):
    if "nc" not in _CACHE:
        _CACHE["nc"] = _build_nc()
    nc = _CACHE["nc"]
    in_maps = _host_prep(**inputs)
    res = run_bass_kernel_spmd(nc, in_maps, core_ids=list(range(NCORES)), trace=trace)
    outs = [res.results[c]["out"] for c in range(NCORES)]
    full = np.concatenate(outs, axis=0).astype(np.float32)
    return full, res


def kernel(**inputs) -> np.ndarray:
    full, _ = run(inputs, trace=False)
    return full



# revision 12
# speedup vs baseline: 2.4572x; 1.2470x over previous
"""HDC generic encoder kernel for 8 Trainium2 NeuronCores.

out[b,d] = sum_{w=0..56} K[w,d] * prod_{j=0..6} enc0[b, w+1+j, (d-(6-j)) mod D]

enc0[b,p,:] is a +/-1 table row selected by level-quantizing x[b,0,p].
Sharding: pure data parallel over batch, 8 batches per core.

Device pipeline per core:
1. 4 indirect row-gathers: R_i[128, 10007] <- extended table rows for
   (b = 2i + p//64, pos = p%64).  One offset per partition (the HW contract).
2. SBUF->SBUF redistribution DMAs into compute layout G[q = b_local*16+blk],
   where each of the 128 partitions holds 64 row-segments of its d-block
   (632 elems = 625 + 6 halo + 1, stride 640), so every rolled factor read
   G[(w+1+j)*SEG + d' + j] is an affine in-partition offset read.
3. Window products via a 5-op tensor_tensor tree (even offsets everywhere
   except the final even*odd combine which runs at 1x):
     E1[p,e]  = G[p*SEG+e]   * G[(p+2)*SEG+e+2]     (factor pairs j, j+2)
     E2[w,d'] = E1[w+1,d']   * E1[w+5,d'+4]         (factors 0,2,4,6)
     O3[w,e]  = E1[w+2,e]    * G[(w+6)*SEG+e+4]     (factors 1,3,5; e=d'+1)
     PROD     = E2[w,d']     * O3[w,d'+1]
     BD       = PROD * keys
4. log-tree adds over w (exact in bf16: +-1 sums <= 57), f32 accumulation
   across w-chunks, one DMA of the [128, 625] accumulator to out [8, 10000].
"""

import numpy as np

import concourse.bacc as bacc
import concourse.bass as bass
import concourse.mybir as mybir
from concourse.bass_utils import run_bass_kernel_spmd
from concourse.tile import TileContext

B, T, F, D = 64, 4, 64, 10000
NGRAMS = 7
W = F - NGRAMS  # 57 windows
NCORES = 8
BPC = B // NCORES  # 8 batches per core
MROWS, HROWS = 3000, 200
VROWS = MROWS + HROWS

NBLK = 16
BLKW = D // NBLK  # 625
SEG = 640  # per-position segment stride in G (even, room for 632)
SEGW = 632  # valid elems per segment (625 + 6 halo + 1)
ET = D + 7  # extended table row width (6 wrap + row + 1)
KW = 626  # keys/intermediate per-window stride (even)

W_E1 = 630
W_E2 = 626
W_O3 = 626
W_PR = 625

CHUNKS = [(0, 8), (8, 8), (16, 8), (24, 8), (32, 8), (40, 8), (48, 8), (56, 1)]

_CACHE = {}


def _build_nc():
    nc = bacc.Bacc(None)
    tbl = nc.dram_tensor(
        "tbl", [VROWS, NBLK * SEG], mybir.dt.bfloat16, kind="ExternalInput"
    )
    keys2 = nc.dram_tensor(
        "keys2", [128, W * KW], mybir.dt.bfloat16, kind="ExternalInput"
    )
    goff = nc.dram_tensor("goff", [128, 4], mybir.dt.int32, kind="ExternalInput")
    out = nc.dram_tensor("out", [BPC, D], mybir.dt.float32, kind="ExternalOutput")
    out_r = out.rearrange("b (q d) -> (b q) d", d=BLKW)  # [128, 625]

    with TileContext(nc) as tc:
        with (
            tc.tile_pool(name="big", bufs=1) as bpool,
            tc.tile_pool(name="rp", bufs=2) as rpool,
            tc.tile_pool(name="work", bufs=1) as wpool,
            tc.tile_pool(name="keysp", bufs=3) as kpool,
        ):
            goff_t = bpool.tile([128, 4], mybir.dt.int32, tag="goff")
            nc.sync.dma_start(out=goff_t[:, :], in_=goff[:, :])

            g = bpool.tile([128, F * SEG], mybir.dt.bfloat16, tag="G")
            g3 = g[:, :].rearrange("p (s k) -> p s k", k=SEG)
            # compute chunk c reads segments w0+1 .. w0+12 only; stream G in
            # three segment ranges so early chunks start during late hop2s
            SRANGES = [(0, 20), (20, 37), (37, 64)]
            with tc.tile_pool(name="scr", bufs=4, space="DRAM") as dpool:
                scrs = []
                for i in (0, 2, 1, 3):
                    # rows pre-blocked in HBM: r[p, blk*SEG+e] = ext_row_p[blk*BLKW+e]
                    r = rpool.tile(
                        [128, NBLK * SEG], mybir.dt.bfloat16, tag="R", name=f"R{i}"
                    )
                    nc.gpsimd.indirect_dma_start(
                        out=r[:, :],
                        out_offset=None,
                        in_=tbl[:, :],
                        in_offset=bass.IndirectOffsetOnAxis(
                            ap=goff_t[:, i : i + 1], axis=0
                        ),
                    )
                    # bounce via DRAM: rows out fat+contiguous, then strided
                    # reads back land [16 partitions, segs, 632] in G layout
                    scr = dpool.tile(
                        [128, NBLK * SEG], mybir.dt.bfloat16, tag="scr", name=f"S{i}"
                    )
                    nc.scalar.dma_start(out=scr[:, :], in_=r[:, :])
                    scrs.append((i, scr))
                for k, (s0, s1) in enumerate(SRANGES):
                    for i, scr in scrs:
                        eng = nc.sync if k == 0 else nc.scalar
                        for h in range(2):
                            p0 = (2 * i + h) * NBLK
                            sv = scr[h * F : (h + 1) * F, :].rearrange(
                                "s (blk e) -> blk s e", e=SEG
                            )
                            eng.dma_start(
                                out=g3[p0 : p0 + NBLK, s0:s1, 0:SEGW].opt(),
                                in_=sv[:, s0:s1, 0:SEGW].opt(),
                            )

            acc = bpool.tile([128, KW], mybir.dt.float32, tag="acc")
            nc.vector.memset(acc[:, :], 0.0)

            def rv(tile, stride, cnt, off, width):
                """Strided-row view: rows of `width` elems at `off + i*stride`."""
                base = (off // stride) * stride
                o2 = off - base
                v = tile[:, base : base + cnt * stride].rearrange(
                    "p (s k) -> p s k", k=stride
                )
                return v[:, :, o2 : o2 + width]

            for w0, wc in CHUNKS:
                kc = kpool.tile([128, wc * KW], mybir.dt.bfloat16, tag="kc")
                nc.sync.dma_start(out=kc[:, :], in_=keys2[:, w0 * KW : (w0 + wc) * KW])

                ne1 = wc + 4
                e1 = wpool.tile([128, ne1 * W_E1], mybir.dt.bfloat16, tag="e1")
                nc.vector.tensor_mul(
                    rv(e1, W_E1, ne1, 0, W_E1),
                    rv(g, SEG, ne1, (w0 + 1) * SEG, W_E1),
                    rv(g, SEG, ne1, (w0 + 3) * SEG + 2, W_E1),
                )
                e2 = wpool.tile([128, wc * KW], mybir.dt.bfloat16, tag="e2")
                nc.vector.tensor_mul(
                    rv(e2, KW, wc, 0, W_E2),
                    rv(e1, W_E1, wc, 0, W_E2),
                    rv(e1, W_E1, wc, 4 * W_E1 + 4, W_E2),
                )
                o3 = wpool.tile([128, wc * KW], mybir.dt.bfloat16, tag="o3")
                nc.vector.tensor_mul(
                    rv(o3, KW, wc, 0, W_O3),
                    rv(e1, W_E1, wc, W_E1, W_O3),
                    rv(g, SEG, wc, (w0 + 6) * SEG + 4, W_O3),
                )
                pr = wpool.tile([128, wc * KW], mybir.dt.bfloat16, tag="e1", name="pr")
                nc.vector.tensor_mul(
                    rv(pr, KW, wc, 0, W_PR),
                    rv(e2, KW, wc, 0, W_PR),
                    rv(o3, KW, wc, 1, W_PR),
                )
                bd = wpool.tile([128, wc * KW], mybir.dt.bfloat16, tag="e2", name="bd")
                nc.vector.tensor_mul(
                    rv(bd, KW, wc, 0, W_PR),
                    rv(pr, KW, wc, 0, W_PR),
                    rv(kc, KW, wc, 0, W_PR),
                )
                n = wc
                while n > 1:
                    m = n // 2
                    nc.vector.tensor_add(
                        rv(bd, KW, m, 0, W_PR),
                        rv(bd, KW, m, 0, W_PR),
                        rv(bd, KW, m, (n - m) * KW, W_PR),
                    )
                    n -= m
                nc.vector.tensor_add(acc[:, 0:W_PR], acc[:, 0:W_PR], bd[:, 0:W_PR])

            nc.sync.dma_start(out=out_r[:, :], in_=acc[:, 0:BLKW])
    nc.compile()
    return nc


def _host_prep(x, keys_weight, motion_table, hr_table):
    import ml_dtypes

    bf16 = ml_dtypes.bfloat16

    import jax.numpy as jnp

    x0 = jnp.asarray(x[:, 0, :])  # [B, F]
    mi = jnp.round((x0[:, : F - 1] - (-3.0)) / (3.0 - (-3.0)) * (MROWS - 1)).astype(
        jnp.int32
    )
    mi = jnp.clip(mi, 0, MROWS - 1)
    hi = jnp.round((x0[:, F - 1] - 50.0) / (200.0 - 50.0) * (HROWS - 1)).astype(
        jnp.int32
    )
    hi = jnp.clip(hi, 0, HROWS - 1) + MROWS
    rows = np.concatenate([np.asarray(mi), np.asarray(hi)[:, None]], axis=1).astype(
        np.int32
    )  # [B, F]

    tb = np.concatenate(
        [np.asarray(motion_table), np.asarray(hr_table)], axis=0
    ).astype(bf16)  # [VROWS, D]
    ext = np.zeros((VROWS, ET), dtype=bf16)
    ext[:, 0:6] = tb[:, D - 6 :]
    ext[:, 6 : 6 + D] = tb
    ext[:, 6 + D] = tb[:, 0]
    # pre-blocked rows with halo baked in: tbl[r, blk*SEG + e] = ext[r, blk*BLKW + e]
    win = np.lib.stride_tricks.sliding_window_view(ext, SEGW, axis=1)  # [V, ET-631, 632]
    tbl = np.zeros((VROWS, NBLK, SEG), dtype=bf16)
    tbl[:, :, 0:SEGW] = win[:, 0 : NBLK * BLKW : BLKW, :]
    tbl = tbl.reshape(VROWS, NBLK * SEG)

    kb = np.asarray(keys_weight)[:W].astype(bf16)  # [57, D]
    karr = np.zeros((NBLK, W, KW), dtype=bf16)
    karr[:, :, :BLKW] = kb.reshape(W, NBLK, BLKW).transpose(1, 0, 2)
    keys2 = np.tile(karr.reshape(NBLK, W * KW), (BPC, 1))  # [128, W*KW]

    in_maps = []
    for c in range(NCORES):
        r8 = rows[BPC * c : BPC * (c + 1)]  # [8, F]
        # goff[p, i] = row index for batch 2i + p//64, pos p%64
        gof = r8.reshape(4, 2 * F).T.copy().astype(np.int32)  # [128, 4]
        in_maps.append({"tbl": tbl, "keys2": keys2, "goff": gof})
    return in_maps


def run(inputs, trace=False):
    if "nc" not in _CACHE:
        _CACHE["nc"] = _build_nc()
    nc = _CACHE["nc"]
    in_maps = _host_prep(**inputs)
    res = run_bass_kernel_spmd(nc, in_maps, core_ids=list(range(NCORES)), trace=trace)
    outs = [res.results[c]["out"] for c in range(NCORES)]
    full = np.concatenate(outs, axis=0).astype(np.float32)
    return full, res


def kernel(**inputs) -> np.ndarray:
    full, _ = run(inputs, trace=False)
    return full

